# revision 30
# baseline (speedup 1.0000x reference)
"""Trainium2 Bass kernel v3 for nn_Attention_42288247996512.

reference:
  q = x @ Wq.T; k = cross @ Wk.T; v = x @ Wv.T
  logits = q @ k.T  (causal; padding m_q*m_k, diag always kept)
  out = softmax(logits / sqrt(128)) @ v

v3: mask-compaction. Keys and queries share the SAME padding mask, so
for unmasked queries the visible keys {k: mask[k]=1, k<=q} are exactly
the causal prefix in mask-compacted coordinates: attention over the
~50% unmasked positions is PURE causal attention in compacted space
(no key/query masks at all except the per-slot diagonal triangle).
Masked queries reduce to out[q] = x[q] @ Wv.T (softmax is one-hot on
the always-kept diagonal): a bare Wv projection ("v-path") that is
batch-independent work, freely assignable to ANY core for balance.

Sharding: 2 cores per batch. The heavy core takes the TOP contiguous
compacted query blocks (largest causal widths), the light core the
bottom blocks; contiguous splits minimize total kT work (the light
core only projects/loads the key prefix it covers). Each core also
gets a v-path quota chosen by water-filling so all 8 cores have equal
PE cycle counts (~101k vs 177k in v2). Attention capacity is padded
to whole 128-blocks; padded rows are masked positions routed through
the attention path with exp-bias -BIG everywhere except +BIG on their
own diagonal, which makes their output exactly v[q] after the host
divide (e(L)/e(L) cancels).

Per-stage structure mirrors v2 (out^T = Wv @ (x^T @ e^T) with the
value projection on the query side), minus all key-mask machinery.
Inputs ride two parallel input queues (SP: ordered attention stream;
Pool/SWDGE: wv, xv, xb); outputs ride ACT. The v-path projection is
do-major so it can start after one 0.25MB wv slice and fills the
front kT/ct DMA window with useful PE work.
"""
import math
import threading

import ml_dtypes
import numpy as np

B, S, D, DA = 4, 2048, 1024, 128
P = 128
KC = D // P  # 8 contraction chunks
NCORES = 8
BIG = 32768.0  # power of two: exactly representable in bf16

_BUILD_LOCK = threading.Lock()
_CACHE: dict = {}


# --------------------------------------------------------------------------
# planning: per-batch split + v-path quota water-fill
# --------------------------------------------------------------------------

def _core_cycles(kmax, nslots, wsum):
    # kT + qT + (logits + transp + AX) in PE rows (== bf16 cycles)
    return 1024 * (kmax + nslots) + 1280 * wsum


def _plan(mask_np):
    """Returns per-core cfg list. Core 2b = heavy(batch b), 2b+1 = light."""
    nbs = mask_np.astype(np.int64).sum(1)  # unmasked per batch
    nblks = [max(2, -(-int(nb) // 128)) for nb in nbs]

    best = None
    import itertools
    for js in itertools.product(*[range(1, nblk) for nblk in nblks]):
        cores = []  # (A_cycles, nslots)
        for nblk, j in zip(nblks, js):
            wh = nblk * (nblk + 1) // 2 - j * (j + 1) // 2
            cores.append((_core_cycles(nblk, nblk - j, wh), nblk - j))
            cores.append((_core_cycles(j, j, j * (j + 1) // 2), j))
        # water-fill Z (columns of Wv-projection work, 64 cyc each)
        zmin = [128 * n for _, n in cores]
        free = list(range(8))
        fixed_z = 0.0
        while True:
            t = (sum(cores[c][0] for c in free) + 64 * (8192 - fixed_z)) \
                / len(free)
            newly = [c for c in free
                     if cores[c][0] + 64 * zmin[c] > t + 1e-9]
            if not newly:
                break
            for c in newly:
                free.remove(c)
                fixed_z += zmin[c]
        obj = max(t, max((cores[c][0] + 64 * zmin[c]
                          for c in range(8) if c not in free), default=0))
        if best is None or obj < best[0]:
            best = (obj, js, t, list(free))
    _, js, t, free = best

    cfgs = []
    zs = []
    for b, (nblk, j) in enumerate(zip(nblks, js)):
        for hv, blocks in ((0, tuple(range(j, nblk))),
                           (1, tuple(range(j)))):
            c = 2 * b + hv
            kmax = max(blocks) + 1
            w = sum(g + 1 for g in blocks)
            a = _core_cycles(kmax, len(blocks), w)
            z = (t - a) / 64.0 if c in free else 128.0 * len(blocks)
            zs.append(max(z, 128.0 * len(blocks)))
            cfgs.append({"batch": b, "blocks": blocks, "kmax": kmax,
                         "nb": int(nbs[b]), "nblk": nblk, "acyc": a})
    # integer Z summing to 8192, symmetric batches kept identical:
    # classes keyed by (nblk, blocks) so equal-shape cores share programs
    zi = [int(z) for z in zs]
    cls = {}
    for c, cfg in enumerate(cfgs):
        cls.setdefault((cfg["nblk"], cfg["blocks"]), []).append(c)
    for members in cls.values():
        zm = min(zi[c] for c in members)
        for c in members:
            zi[c] = zm
    # distribute the remainder: bump whole classes (smallest first) while
    # possible, then dump the tail on the first core of the smallest class
    # (that core then just compiles its own program variant)
    rem = 8192 - sum(zi)
    order = sorted(cls.values(), key=len)
    progress = True
    while rem > 0 and progress:
        progress = False
        for members in order:
            if rem >= len(members):
                for c in members:
                    zi[c] += 1
                rem -= len(members)
                progress = True
    if rem > 0:
        zi[order[0][0]] += rem
    for c, cfg in enumerate(cfgs):
        cfg["vcols"] = zi[c] - 128 * len(cfg["blocks"])
        assert cfg["vcols"] >= 0
    assert sum(zi) == 8192
    return cfgs


# --------------------------------------------------------------------------
# kernel builder (one program per distinct (kmax, blocks, vcols))
# --------------------------------------------------------------------------

def _build(kmax, blocks, vcols):
    from contextlib import ExitStack

    import concourse.mybir as mybir
    import concourse.tile as tile
    from concourse import bacc
    from concourse.masks import make_identity

    dt = mybir.dt
    f32 = dt.float32
    bf16 = dt.bfloat16
    AF = mybir.ActivationFunctionType
    ALU = mybir.AluOpType

    nslots = len(blocks)
    K = kmax * P          # keys held on this core
    QA = nslots * P       # attention query columns
    Z = QA + vcols        # total output columns
    # adjacent-slot pairs (+ possibly a lone last slot)
    pairs = [tuple(range(i, min(i + 2, nslots))) for i in range(0, nslots, 2)]
    # per-slot key coverage in 128-blocks (exact causal in compacted space)
    nkb = [g + 1 for g in blocks]

    nc = bacc.Bacc("TRN2", target_bir_lowering=False, debug=False)

    ct_d = nc.dram_tensor("ct", [P, KC, K], bf16, kind="ExternalInput").ap()
    xq_d = nc.dram_tensor("xq", [P, KC, QA], bf16, kind="ExternalInput").ap()
    if vcols:
        xv_d = nc.dram_tensor("xv", [P, KC, vcols], bf16,
                              kind="ExternalInput").ap()
    wq_d = nc.dram_tensor("wq", [P, KC, DA], bf16, kind="ExternalInput").ap()
    wk_d = nc.dram_tensor("wk", [P, KC, DA], bf16, kind="ExternalInput").ap()
    wv_d = nc.dram_tensor("wv", [P, KC, KC, P], bf16,
                          kind="ExternalInput").ap()  # [p, do, kc, 128]
    xb_d = nc.dram_tensor("xb", [K, D], bf16, kind="ExternalInput").ap()
    qmn_d = nc.dram_tensor("qmn", [P, nslots], f32, kind="ExternalInput").ap()
    dm_d = nc.dram_tensor("dmask", [P, QA], bf16, kind="ExternalInput").ap()

    outT = nc.dram_tensor("outT", [D, Z], bf16, kind="ExternalOutput").ap()
    den_d = nc.dram_tensor("den", [P, nslots], f32, kind="ExternalOutput").ap()

    xb_r = xb_d.rearrange("(g p) d -> p g d", p=P)
    outT_r = outT.rearrange("(do p) q -> p do q", p=P)

    with tile.TileContext(nc) as tc, ExitStack() as ctx:
        const = ctx.enter_context(tc.tile_pool(name="const", bufs=1))
        persist = ctx.enter_context(tc.tile_pool(name="persist", bufs=1))
        apool = ctx.enter_context(tc.tile_pool(name="apool", bufs=4))
        epool = ctx.enter_context(tc.tile_pool(name="epool", bufs=16))
        psl_pool = ctx.enter_context(tc.tile_pool(name="psl", bufs=2,
                                                  space="PSUM"))
        psT_pool = ctx.enter_context(tc.tile_pool(name="psT", bufs=2,
                                                  space="PSUM"))
        psax_pool = ctx.enter_context(tc.tile_pool(name="psax", bufs=2,
                                                   space="PSUM"))
        pjp_pool = ctx.enter_context(tc.tile_pool(name="pjp", bufs=2,
                                                  space="PSUM"))

        ident_f32 = const.tile([P, P], f32, name="ident_f32")
        make_identity(nc, ident_f32)
        ident = const.tile([P, P], bf16, name="ident")
        nc.vector.tensor_copy(ident[:], ident_f32[:])
        # touch Exp once so ACT loads its function table during the front
        # DMA window instead of on the first logits chunk
        scratch = const.tile([1, 1], f32, name="scratch")
        nc.scalar.activation(scratch[:], ident_f32[0:1, 0:1], AF.Exp)

        wq_sb = const.tile([P, KC, DA], bf16, name="wq_sb")
        wk_sb = const.tile([P, KC, DA], bf16, name="wk_sb")
        wv_sb = const.tile([P, KC, KC, P], bf16, name="wv_sb")
        qmn_sb = const.tile([P, nslots], f32, name="qmn_sb")
        dm_sb = const.tile([P, QA], bf16, name="dm_sb")

        kT_sb = persist.tile([P, K], bf16, name="kT_sb")
        qT_sb = persist.tile([P, QA], bf16, name="qT_sb")
        ct_sb = persist.tile([P, KC, K], bf16, name="ct_sb")
        xq_sb = persist.tile([P, KC, QA], bf16, name="xq_sb")
        if vcols:
            xv_sb = persist.tile([P, KC, vcols], bf16, name="xv_sb")
            vosb = persist.tile([P, KC, vcols], bf16, name="vosb")
        xb_sb = persist.tile([P, kmax, D], bf16, name="xb_sb")
        den_sb = persist.tile([P, nslots], f32, name="den_sb")

        es: dict = {}    # (slot, kb128) -> (tile, col_offset)
        dacs: dict = {s: [] for s in range(nslots)}

        # ---- stage emitters ------------------------------------------------

        # kT chunks of 256 keys (+ partial tail): fine granularity so the
        # first matmuls start ~2.5us earlier while ct still streams in
        kcw = [(o, min(256, K - o)) for o in range(0, K, 256)]

        def kT_dma(ci):
            o, w = kcw[ci]
            nc.sync.dma_start(ct_sb[:, :, o:o + w], ct_d[:, :, o:o + w])

        def kT_mm(ci):
            o, w = kcw[ci]
            ps = psl_pool.tile([P, 512], f32, tag="psl", name=f"psk{ci}")
            for kc in range(KC):
                nc.tensor.matmul(ps[:, :w], lhsT=wk_sb[:, kc, :],
                                 rhs=ct_sb[:, kc, o:o + w],
                                 start=(kc == 0), stop=(kc == KC - 1))
            nc.vector.tensor_copy(kT_sb[:, o:o + w], ps[:, :w])

        # qT chunks of 256 query cols (+ partial tail)
        qcw = [(o, min(256, QA - o)) for o in range(0, QA, 256)]

        def qT_dma(ci, eng=None):
            o, w = qcw[ci]
            (eng or nc.sync).dma_start(xq_sb[:, :, o:o + w],
                                       xq_d[:, :, o:o + w])

        def qT_mm(ci):
            o, w = qcw[ci]
            ps = psl_pool.tile([P, 512], f32, tag="psl", name=f"psq{ci}")
            for kc in range(KC):
                nc.tensor.matmul(ps[:, :w], lhsT=wq_sb[:, kc, :],
                                 rhs=xq_sb[:, kc, o:o + w],
                                 start=(kc == 0), stop=(kc == KC - 1))
            nc.vector.tensor_copy(qT_sb[:, o:o + w], ps[:, :w])

        # v-path projection, do-major: per do, one psum group per <=512 cols
        if vcols:
            if vcols > 512:
                h = (vcols + 1) // 2
                vgroups = [(0, h), (h, vcols - h)]
            else:
                vgroups = [(0, vcols)]

        vp_flushed = [0]

        def vp_item(do, gi):
            o, w = vgroups[gi]
            ps = pjp_pool.tile([P, 512], f32, tag="pjp", name=f"psv{do}_{o}")
            for kc in range(KC):
                nc.tensor.matmul(ps[:, :w], lhsT=wv_sb[:, do, kc, :],
                                 rhs=xv_sb[:, kc, o:o + w],
                                 start=(kc == 0), stop=(kc == KC - 1))
            nc.gpsimd.tensor_copy(vosb[:, do, o:o + w], ps[:, :w])
            # flush completed dos in pairs so the output DMA overlaps the
            # remaining compute (fillers pop in do order)
            if gi == len(vgroups) - 1 and (do % 2 == 1 or do == KC - 1):
                d0 = vp_flushed[0]
                nc.gpsimd.dma_start(outT_r[:, d0:do + 1, QA:],
                                    vosb[:, d0:do + 1, :])
                vp_flushed[0] = do + 1

        # logits+exp for one chunk of one slot. ACT reads the PSUM bank
        # directly; the chunk containing the slot's diagonal 128-block first
        # gets the triangular/pad mask added in-place on DVE.
        def slot_logit_chunk(s, o, w):
            wtot = nkb[s] * P
            psl = psl_pool.tile([P, 512], f32, tag="psl",
                                name=f"psl{s}_{o}")
            nc.tensor.matmul(psl[:, :w], lhsT=qT_sb[:, s * P:(s + 1) * P],
                             rhs=kT_sb[:, o:o + w], start=True, stop=True)
            e = apool.tile([P, 512], bf16, tag="e", name=f"e{s}_{o}",
                           bufs=10)
            dac = apool.tile([P, 1], f32, tag="dac", name=f"dac{s}_{o}",
                             bufs=12)
            if o + w == wtot:  # this chunk ends with the diagonal block
                dc = w - P
                nc.vector.tensor_tensor(
                    out=psl[:, dc:dc + P], in0=psl[:, dc:dc + P],
                    in1=dm_sb[:, s * P:(s + 1) * P], op=ALU.add)
            nc.scalar.activation(e[:, :w], psl[:, :w], AF.Exp,
                                 bias=qmn_sb[:, s:s + 1], scale=1.0,
                                 accum_out=dac[:])
            for ki in range(w // P):
                es[(s, o // P + ki)] = (e, ki * P)
            dacs[s].append(dac)

        # transpose a pair's e blocks; 2 key-blocks share one psT bank/eT
        def pair_transp(pr):
            sl = pairs[pr]
            both_n = nkb[sl[0]] if len(sl) == 2 else 0
            top_n = nkb[sl[-1]]
            eTs = []
            for kh in range((top_n + 1) // 2):
                psT = psT_pool.tile([P, 512], bf16, tag="psT",
                                    name=f"psT{pr}_{kh}")
                eT = epool.tile([P, 512], bf16, tag="eT", name=f"eT{pr}_{kh}")
                runs = []
                for ki in range(2):
                    kb = 2 * kh + ki
                    if kb >= top_n:
                        break
                    base = ki * 256
                    if kb < both_n:
                        t0, o0 = es[(sl[0], kb)]
                        nc.tensor.transpose(psT[:, base:base + P],
                                            t0[:, o0:o0 + P], ident[:])
                        t1, o1 = es[(sl[1], kb)]
                        nc.tensor.transpose(psT[:, base + P:base + 2 * P],
                                            t1[:, o1:o1 + P], ident[:])
                        w = 256
                    else:
                        t1, o1 = es[(sl[-1], kb)]
                        nc.tensor.transpose(psT[:, base:base + P],
                                            t1[:, o1:o1 + P], ident[:])
                        w = P
                    if runs and runs[-1][0] + runs[-1][1] == base:
                        runs[-1] = (runs[-1][0], runs[-1][1] + w)
                    else:
                        runs.append((base, w))
                    eTs.append((eT, base, kb < both_n))
                for bse, w in runs:
                    nc.vector.tensor_copy(eT[:, bse:bse + w], psT[:, bse:bse + w])
            return eTs

        # AX: zT[dm, q] = sum_k x[k, dm] * e[q, k] over the pair's coverage.
        # One item = one psum bank (512 f32) = dpb dm-chunks, a ~0.5-1.5us
        # filler unit.
        def ax_item(pr, zT, eTs, dmh):
            sl = pairs[pr]
            qw = 128 * len(sl)          # 256 for pairs, 128 for lone slot
            dpb = 512 // qw             # dm-chunks packed per psum bank
            top_n = nkb[sl[-1]]
            ps = psax_pool.tile([P, 512], f32, tag="psax",
                                name=f"psax{pr}_{dmh}")
            for sub in range(dpb):
                dmc = dmh * dpb + sub
                base = sub * qw
                for kb in range(top_n):
                    eT, eb, both = eTs[kb]
                    lhsT = xb_sb[:, kb, dmc * P:(dmc + 1) * P]
                    first = (kb == 0 and sub == 0)
                    last = (kb == top_n - 1 and sub == dpb - 1)
                    if both:
                        # blocks are distinct ascending, so the last kb of a
                        # pair always falls in the lone-slot branch below
                        nc.tensor.matmul(ps[:, base:base + P], lhsT=lhsT,
                                         rhs=eT[:, eb:eb + P],
                                         start=first, stop=False)
                        nc.tensor.matmul(
                            ps[:, base + P:base + 2 * P], lhsT=lhsT,
                            rhs=eT[:, eb + P:eb + 2 * P],
                            start=False, stop=False)
                    else:
                        # pair: the surviving blocks belong to the top slot
                        # (second 128 cols); lone slot: the only 128 cols
                        b0 = base + (P if len(sl) == 2 else 0)
                        nc.tensor.matmul(
                            ps[:, b0:b0 + P], lhsT=lhsT,
                            rhs=eT[:, eb:eb + P],
                            start=first, stop=last)
            nc.gpsimd.tensor_copy(zT[:, dmh * dpb:(dmh + 1) * dpb, :], ps[:])

        def pair_den(pr):
            for s in pairs[pr]:
                dl = dacs[s]
                dst = den_sb[:, s:s + 1]
                if len(dl) == 1:
                    nc.vector.tensor_copy(dst, dl[0][:])
                else:
                    nc.vector.tensor_tensor(out=dst, in0=dl[0][:],
                                            in1=dl[1][:], op=ALU.add)
                    for d in dl[2:]:
                        nc.vector.tensor_tensor(out=dst, in0=dst, in1=d[:],
                                                op=ALU.add)

        # attention-side projection item: one dout of one pair's zT.
        # All pairs share one osb staging tile; flushes merge adjacent pairs
        # so every output descriptor is >= 512B (no small-desc penalty).
        osb = persist.tile([P, KC, QA], bf16, name="osb")
        osb_flushed = [0]

        def proj_item(pr, zT, do):
            sl = pairs[pr]
            qw = 128 * len(sl)
            q0 = sl[0] * P
            ps = pjp_pool.tile([P, 512], f32, tag="pjp", name=f"psp{pr}_{do}")
            for dmc in range(KC):
                nc.tensor.matmul(ps[:, :qw], lhsT=wv_sb[:, do, dmc, :],
                                 rhs=zT[:, dmc, :],
                                 start=(dmc == 0), stop=(dmc == KC - 1))
            nc.gpsimd.tensor_copy(osb[:, do, q0:q0 + qw], ps[:, :qw])
            if do == KC - 1:
                f0 = osb_flushed[0]
                unflushed = q0 + qw - f0
                remaining = QA - (q0 + qw)
                if remaining == 0 or (unflushed >= 256 and remaining >= 256):
                    nc.gpsimd.dma_start(outT_r[:, :, f0:f0 + unflushed],
                                        osb[:, :, f0:f0 + unflushed])
                    osb_flushed[0] = q0 + qw

        # ---- DMA queue -----------------------------------------------------
        # The cost model serializes ALL transfers through one DMA-engines
        # resource (360B/ns, 2x penalty under 512B runs), so parallel queues
        # buy nothing: put every input on the SP HWDGE queue in exact PE
        # consumption order, and outputs on the ACT queue (they must not
        # block later inputs).
        kt_need_pr = [-(-nkb[pairs[pr][-1]] * P // 256)
                      for pr in range(len(pairs))]
        heavy = blocks[0] != 0
        xb_grp = []
        xb_done = 0
        for pr in range(len(pairs)):
            need = nkb[pairs[pr][-1]]
            if need > xb_done:
                xb_grp.append((pr, xb_done, need))
                xb_done = need

        nc.sync.dma_start(wq_sb[:], wq_d)
        qT_dma(0)
        nc.sync.dma_start(wk_sb[:], wk_d)
        # light cores live off v-path fillers from the start: the first xv
        # group + first wv slices lead; heavy cores want ct first
        front_vp = vcols >= 256 and not heavy
        if front_vp:
            nc.sync.dma_start(wv_sb[:, 0], wv_d[:, 0])
            o, w = vgroups[0]
            nc.sync.dma_start(xv_sb[:, :, o:o + w], xv_d[:, :, o:o + w])
            nc.sync.dma_start(wv_sb[:, 1], wv_d[:, 1])
            wv_rest = list(range(2, KC))
        else:
            wv_rest = list(range(KC))
        kd = 0
        for pr in range(len(pairs)):
            while kd < kt_need_pr[pr]:
                kT_dma(kd)
                kd += 1
            if pr == 0:
                nc.sync.dma_start(qmn_sb[:], qmn_d)
                nc.sync.dma_start(dm_sb[:], dm_d)
                if vcols and not front_vp:
                    nc.sync.dma_start(wv_sb[:, 0], wv_d[:, 0])
                    for o, w in vgroups:
                        nc.sync.dma_start(xv_sb[:, :, o:o + w],
                                          xv_d[:, :, o:o + w])
                    nc.sync.dma_start(wv_sb[:, 1], wv_d[:, 1])
                    wv_rest = list(range(2, KC))
                elif front_vp and len(vgroups) > 1:
                    for o, w in vgroups[1:]:
                        nc.sync.dma_start(xv_sb[:, :, o:o + w],
                                          xv_d[:, :, o:o + w])
            if pr + 1 < len(qcw):
                qT_dma(pr + 1)
            # xb group consumed by AX(pr-1) during THIS pair's logits
            for g, a, b_ in xb_grp:
                if g == pr - 1:
                    nc.sync.dma_start(xb_sb[:, a:b_, :], xb_r[:, a:b_, :])
            for _ in range(3 if heavy else 2):
                if wv_rest:
                    do = wv_rest.pop(0)
                    nc.sync.dma_start(wv_sb[:, do], wv_d[:, do])
        for g, a, b_ in xb_grp:
            if g >= len(pairs) - 1:
                nc.sync.dma_start(xb_sb[:, a:b_, :], xb_r[:, a:b_, :])
        while wv_rest:
            do = wv_rest.pop(0)
            nc.sync.dma_start(wv_sb[:, do], wv_d[:, do])

        # ---- PE schedule ---------------------------------------------------
        # Emission order == per-engine execution order. "Fillers" are
        # dependency-free ~1us PE work items (v-path dos, AX of finished
        # pairs, projections) spliced between logits chunks / transposes so
        # the PE never waits on the DVE/ACT exp chain or PSUM turnarounds.
        from collections import deque
        fillers: deque = deque()

        def drain(n):
            while fillers and n > 0:
                fillers.popleft()()
                n -= 1

        # group-major: all dos of xv group 0 first (group 1's DMA lands
        # later on light cores)
        vp_seq = [(do, gi) for gi in range(len(vgroups))
                  for do in range(KC)] if vcols else []
        nfront = min(len(vp_seq), 4) if front_vp else 0
        for it in vp_seq[:nfront]:
            fillers.append(lambda it=it: vp_item(*it))
        vp_rest = deque(vp_seq[nfront:])

        # front: qT0 (its inputs lead the queue), then kT chunks chasing
        # the ct DMA stream with fillers between
        qT_mm(0)
        kt_done = 0
        while kt_done < kt_need_pr[0]:
            kT_mm(kt_done)
            kt_done += 1
            drain(1)
        drain(max(0, len(fillers) - 1))

        zTs = {}
        for pr in range(len(pairs)):
            if pr + 1 < len(qcw):
                qT_mm(pr + 1)
            while kt_done < kt_need_pr[pr]:
                kT_mm(kt_done)
                kt_done += 1
            # logits chunks with a filler after every second chunk
            nch = 0
            for s in pairs[pr]:
                wtot = nkb[s] * P
                for o in range(0, wtot, 512):
                    slot_logit_chunk(s, o, min(512, wtot - o))
                    nch += 1
                    if nch % 2 == 0:
                        drain(1)
            drain(1)
            eTs = pair_transp(pr)
            pair_den(pr)
            # queue this pair's AX; its completion queues the projections
            sl = pairs[pr]
            qw = 128 * len(sl)
            dpb = 512 // qw
            zT = apool.tile([P, KC, qw], bf16, tag="zT", name=f"zT{pr}",
                            bufs=4)
            zTs[pr] = zT

            def make_ax(pr=pr, zT=zT, eTs=eTs, dpb=dpb):
                def run(dmh):
                    ax_item(pr, zT, eTs, dmh)
                    if dmh == KC // dpb - 1:
                        for do in range(KC):
                            fillers.append(
                                lambda do=do: proj_item(pr, zT, do))
                return run
            ax_run = make_ax()
            # interleave v-path items between AX items: AX may briefly wait
            # on its xb group, and the v-path keeps the PE fed meanwhile
            for dmh in range(KC // dpb):
                if vp_rest and dmh % 2 == 0:
                    fillers.append(
                        lambda it=vp_rest.popleft(): vp_item(*it))
                fillers.append(lambda dmh=dmh, ax_run=ax_run: ax_run(dmh))

        nc.gpsimd.dma_start(den_d, den_sb[:])
        # tail: drain AX/proj fillers and leftover v-path items
        while fillers or vp_rest:
            if fillers:
                fillers.popleft()()
            elif vp_rest:
                vp_item(*vp_rest.popleft())

    nc.compile()
    return nc


def _get_programs(mask_np):
    key = mask_np.tobytes()
    with _BUILD_LOCK:
        if _CACHE.get("key") != key:
            cfgs = _plan(mask_np)
            progs = {}
            for cfg in cfgs:
                sig = (cfg["kmax"], cfg["blocks"], cfg["vcols"])
                if sig not in progs:
                    progs[sig] = _build(*sig)
            _CACHE.update(key=key, cfgs=cfgs, progs=progs)
        return _CACHE["cfgs"], _CACHE["progs"]


def _get_ncs():
    return tuple(_CACHE["progs"].values())


# --------------------------------------------------------------------------
# host side
# --------------------------------------------------------------------------

def make_in_maps(x, cross, Wq, Wk, Wv, mask, cfgs):
    bf = ml_dtypes.bfloat16
    x = np.asarray(x, dtype=np.float32)
    cross = np.asarray(cross, dtype=np.float32)
    mask_np = np.asarray(mask)
    scale = 1.0 / math.sqrt(DA)
    wq_h = np.ascontiguousarray(
        (np.asarray(Wq, np.float32) * scale).T.reshape(KC, P, DA)
        .transpose(1, 0, 2)).astype(bf)
    wk_h = np.ascontiguousarray(
        np.asarray(Wk, np.float32).T.reshape(KC, P, DA)
        .transpose(1, 0, 2)).astype(bf)
    # wv do-major: wv_h[p, do, kc, j] = Wv[do*128+j, kc*128+p]
    wv_h = np.ascontiguousarray(
        np.asarray(Wv, np.float32).T.reshape(KC, P, KC, P)
        .transpose(1, 2, 0, 3)).astype(bf)

    # per-batch compacted order: unmasked asc, then masked asc
    orders, caps = [], []
    for b in range(B):
        m = mask_np[b].astype(bool)
        un = np.flatnonzero(m)
        ma = np.flatnonzero(~m)
        cap = cfgs[2 * b]["nblk"] * P
        orders.append(np.concatenate([un, ma[:cap - len(un)]]))
        caps.append(cap)
    # global leftover v-pool: (batch, q) pairs
    vpool = []
    for b in range(B):
        m = mask_np[b].astype(bool)
        ma = np.flatnonzero(~m)
        for q in ma[caps[b] - int(m.sum()):]:
            vpool.append((b, int(q)))
    assert len(vpool) == sum(c["vcols"] for c in cfgs)

    in_maps, metas = [], []
    vo = 0
    for c, cfg in enumerate(cfgs):
        b = cfg["batch"]
        blocks = cfg["blocks"]
        kmax, nb = cfg["kmax"], cfg["nb"]
        order = orders[b]
        K = kmax * P
        qpos = np.concatenate([order[g * P:(g + 1) * P] for g in blocks])
        vlist = vpool[vo:vo + cfg["vcols"]]
        vo += cfg["vcols"]

        ct_h = np.ascontiguousarray(
            cross[b][order[:K]].T.reshape(KC, P, K)
            .transpose(1, 0, 2)).astype(bf)
        xb_h = np.ascontiguousarray(x[b][order[:K]]).astype(bf)
        xq_h = np.ascontiguousarray(
            x[b][qpos].T.reshape(KC, P, len(qpos))
            .transpose(1, 0, 2)).astype(bf)

        qmn_h = np.zeros((P, len(blocks)), np.float32)
        dm_h = np.zeros((P, len(blocks) * P), np.float32)
        rows = np.arange(P)
        for s, g in enumerate(blocks):
            padded = g * P + rows >= nb          # [P] bool
            qmn_h[padded, s] = -BIG
            tri = np.where(rows[None, :] <= rows[:, None], 0.0, -BIG)
            pad_row = np.where(rows[None, :] == rows[:, None], BIG, -BIG)
            dm_h[:, s * P:(s + 1) * P] = np.where(padded[:, None],
                                                  pad_row, tri)
        im = {"ct": ct_h, "xq": xq_h, "wq": wq_h, "wk": wk_h,
              "wv": wv_h, "xb": xb_h, "qmn": qmn_h,
              "dmask": dm_h.astype(bf)}
        if cfg["vcols"]:
            xv_rows = np.stack([x[bb, qq] for bb, qq in vlist])
            im["xv"] = np.ascontiguousarray(
                xv_rows.T.reshape(KC, P, len(vlist))
                .transpose(1, 0, 2)).astype(bf)
        in_maps.append(im)
        metas.append({"batch": b, "qpos": qpos, "vlist": vlist,
                      "nslots": len(blocks)})
    return in_maps, metas


def kernel(x, cross, Wq, Wk, Wv, mask):
    from concourse import bass_utils

    mask_np = np.asarray(mask)
    cfgs, progs = _get_programs(mask_np)
    in_maps, metas = make_in_maps(x, cross, Wq, Wk, Wv, mask, cfgs)

    # group cores by program
    groups: dict = {}
    for c, cfg in enumerate(cfgs):
        sig = (cfg["kmax"], cfg["blocks"], cfg["vcols"])
        groups.setdefault(sig, []).append(c)

    results = {}
    for sig, cores in groups.items():
        res = bass_utils.run_bass_kernel_spmd(
            progs[sig], [in_maps[c] for c in cores], core_ids=cores)
        for i, c in enumerate(cores):
            results[c] = res.results[i]

    out = np.empty((B, S, D), np.float32)
    for c, meta in enumerate(metas):
        r = results[c]
        o = r["outT"].astype(np.float32).T  # [Z, 1024]
        qa = len(meta["qpos"])
        denf = r["den"].T.reshape(-1).astype(np.float32)  # [QA] slot-major
        out[meta["batch"], meta["qpos"]] = o[:qa] / denf[:, None]
        for i, (bb, qq) in enumerate(meta["vlist"]):
            out[bb, qq] = o[qa + i]
    return out


# revision 31
# speedup vs baseline: 1.0297x; 1.0297x over previous
"""Trainium2 Bass kernel v3 for nn_Attention_42288247996512.

reference:
  q = x @ Wq.T; k = cross @ Wk.T; v = x @ Wv.T
  logits = q @ k.T  (causal; padding m_q*m_k, diag always kept)
  out = softmax(logits / sqrt(128)) @ v

v3: mask-compaction. Keys and queries share the SAME padding mask, so
for unmasked queries the visible keys {k: mask[k]=1, k<=q} are exactly
the causal prefix in mask-compacted coordinates: attention over the
~50% unmasked positions is PURE causal attention in compacted space
(no key/query masks at all except the per-slot diagonal triangle).
Masked queries reduce to out[q] = x[q] @ Wv.T (softmax is one-hot on
the always-kept diagonal): a bare Wv projection ("v-path") that is
batch-independent work, freely assignable to ANY core for balance.

Sharding: 2 cores per batch. The heavy core takes the TOP contiguous
compacted query blocks (largest causal widths), the light core the
bottom blocks; contiguous splits minimize total kT work (the light
core only projects/loads the key prefix it covers). Each core also
gets a v-path quota chosen by water-filling so all 8 cores have equal
PE cycle counts (~101k vs 177k in v2). Attention capacity is padded
to whole 128-blocks; padded rows are masked positions routed through
the attention path with exp-bias -BIG everywhere except +BIG on their
own diagonal, which makes their output exactly v[q] after the host
divide (e(L)/e(L) cancels).

Per-stage structure mirrors v2 (out^T = Wv @ (x^T @ e^T) with the
value projection on the query side), minus all key-mask machinery.
Inputs ride two parallel input queues (SP: ordered attention stream;
Pool/SWDGE: wv, xv, xb); outputs ride ACT. The v-path projection is
do-major so it can start after one 0.25MB wv slice and fills the
front kT/ct DMA window with useful PE work.
"""
import math
import threading

import ml_dtypes
import numpy as np

B, S, D, DA = 4, 2048, 1024, 128
P = 128
KC = D // P  # 8 contraction chunks
NCORES = 8
BIG = 32768.0  # power of two: exactly representable in bf16

_BUILD_LOCK = threading.Lock()
_CACHE: dict = {}


# --------------------------------------------------------------------------
# planning: per-batch split + v-path quota water-fill
# --------------------------------------------------------------------------

def _core_cycles(kmax, nslots, wsum):
    # kT + qT + (logits + transp + AX) in PE rows (== bf16 cycles)
    return 1024 * (kmax + nslots) + 1280 * wsum


def _plan(mask_np):
    """Returns per-core cfg list. Core 2b = heavy(batch b), 2b+1 = light."""
    nbs = mask_np.astype(np.int64).sum(1)  # unmasked per batch
    nblks = [max(2, -(-int(nb) // 128)) for nb in nbs]

    best = None
    import itertools
    for js in itertools.product(*[range(1, nblk) for nblk in nblks]):
        cores = []  # (A_cycles, nslots)
        for nblk, j in zip(nblks, js):
            wh = nblk * (nblk + 1) // 2 - j * (j + 1) // 2
            cores.append((_core_cycles(nblk, nblk - j, wh), nblk - j))
            cores.append((_core_cycles(j, j, j * (j + 1) // 2), j))
        # water-fill Z (columns of Wv-projection work, 64 cyc each)
        zmin = [128 * n for _, n in cores]
        free = list(range(8))
        fixed_z = 0.0
        while True:
            t = (sum(cores[c][0] for c in free) + 64 * (8192 - fixed_z)) \
                / len(free)
            newly = [c for c in free
                     if cores[c][0] + 64 * zmin[c] > t + 1e-9]
            if not newly:
                break
            for c in newly:
                free.remove(c)
                fixed_z += zmin[c]
        obj = max(t, max((cores[c][0] + 64 * zmin[c]
                          for c in range(8) if c not in free), default=0))
        if best is None or obj < best[0]:
            best = (obj, js, t, list(free))
    _, js, t, free = best

    cfgs = []
    zs = []
    for b, (nblk, j) in enumerate(zip(nblks, js)):
        for hv, blocks in ((0, tuple(range(j, nblk))),
                           (1, tuple(range(j)))):
            c = 2 * b + hv
            kmax = max(blocks) + 1
            w = sum(g + 1 for g in blocks)
            a = _core_cycles(kmax, len(blocks), w)
            z = (t - a) / 64.0 if c in free else 128.0 * len(blocks)
            zs.append(max(z, 128.0 * len(blocks)))
            cfgs.append({"batch": b, "blocks": blocks, "kmax": kmax,
                         "nb": int(nbs[b]), "nblk": nblk, "acyc": a})
    # integer Z summing to 8192, symmetric batches kept identical:
    # classes keyed by (nblk, blocks) so equal-shape cores share programs
    zi = [int(z) for z in zs]
    cls = {}
    for c, cfg in enumerate(cfgs):
        cls.setdefault((cfg["nblk"], cfg["blocks"]), []).append(c)
    for members in cls.values():
        zm = min(zi[c] for c in members)
        for c in members:
            zi[c] = zm
    # distribute the remainder: bump whole classes (smallest first) while
    # possible, then dump the tail on the first core of the smallest class
    # (that core then just compiles its own program variant)
    rem = 8192 - sum(zi)
    order = sorted(cls.values(), key=len)
    progress = True
    while rem > 0 and progress:
        progress = False
        for members in order:
            if rem >= len(members):
                for c in members:
                    zi[c] += 1
                rem -= len(members)
                progress = True
    if rem > 0:
        zi[order[0][0]] += rem
    for c, cfg in enumerate(cfgs):
        cfg["vcols"] = zi[c] - 128 * len(cfg["blocks"])
        assert cfg["vcols"] >= 0
    assert sum(zi) == 8192
    return cfgs


# --------------------------------------------------------------------------
# kernel builder (one program per distinct (kmax, blocks, vcols))
# --------------------------------------------------------------------------

def _build(kmax, blocks, vcols):
    from contextlib import ExitStack

    import concourse.mybir as mybir
    import concourse.tile as tile
    from concourse import bacc
    from concourse.masks import make_identity

    dt = mybir.dt
    f32 = dt.float32
    bf16 = dt.bfloat16
    AF = mybir.ActivationFunctionType
    ALU = mybir.AluOpType

    nslots = len(blocks)
    K = kmax * P          # keys held on this core
    QA = nslots * P       # attention query columns
    Z = QA + vcols        # total output columns
    # adjacent-slot pairs (+ possibly a lone last slot)
    pairs = [tuple(range(i, min(i + 2, nslots))) for i in range(0, nslots, 2)]
    # per-slot key coverage in 128-blocks (exact causal in compacted space)
    nkb = [g + 1 for g in blocks]

    nc = bacc.Bacc("TRN2", target_bir_lowering=False, debug=False)

    ct_d = nc.dram_tensor("ct", [P, KC, K], bf16, kind="ExternalInput").ap()
    xq_d = nc.dram_tensor("xq", [P, KC, QA], bf16, kind="ExternalInput").ap()
    if vcols:
        xv_d = nc.dram_tensor("xv", [P, KC, vcols], bf16,
                              kind="ExternalInput").ap()
    wq_d = nc.dram_tensor("wq", [P, KC, DA], bf16, kind="ExternalInput").ap()
    wk_d = nc.dram_tensor("wk", [P, KC, DA], bf16, kind="ExternalInput").ap()
    wv_d = nc.dram_tensor("wv", [P, KC, KC, P], bf16,
                          kind="ExternalInput").ap()  # [p, do, kc, 128]
    xb_d = nc.dram_tensor("xb", [K, D], bf16, kind="ExternalInput").ap()
    qmn_d = nc.dram_tensor("qmn", [P, nslots], f32, kind="ExternalInput").ap()
    dm_d = nc.dram_tensor("dmask", [P, QA], bf16, kind="ExternalInput").ap()

    outT = nc.dram_tensor("outT", [D, Z], bf16, kind="ExternalOutput").ap()
    den_d = nc.dram_tensor("den", [P, nslots], f32, kind="ExternalOutput").ap()

    xb_r = xb_d.rearrange("(g p) d -> p g d", p=P)
    outT_r = outT.rearrange("(do p) q -> p do q", p=P)

    with tile.TileContext(nc) as tc, ExitStack() as ctx:
        const = ctx.enter_context(tc.tile_pool(name="const", bufs=1))
        persist = ctx.enter_context(tc.tile_pool(name="persist", bufs=1))
        apool = ctx.enter_context(tc.tile_pool(name="apool", bufs=4))
        epool = ctx.enter_context(tc.tile_pool(name="epool", bufs=16))
        psl_pool = ctx.enter_context(tc.tile_pool(name="psl", bufs=2,
                                                  space="PSUM"))
        psT_pool = ctx.enter_context(tc.tile_pool(name="psT", bufs=2,
                                                  space="PSUM"))
        psax_pool = ctx.enter_context(tc.tile_pool(name="psax", bufs=2,
                                                   space="PSUM"))
        pjp_pool = ctx.enter_context(tc.tile_pool(name="pjp", bufs=2,
                                                  space="PSUM"))

        ident_f32 = const.tile([P, P], f32, name="ident_f32")
        make_identity(nc, ident_f32)
        ident = const.tile([P, P], bf16, name="ident")
        nc.vector.tensor_copy(ident[:], ident_f32[:])
        # touch Exp once so ACT loads its function table during the front
        # DMA window instead of on the first logits chunk
        scratch = const.tile([1, 1], f32, name="scratch")
        nc.scalar.activation(scratch[:], ident_f32[0:1, 0:1], AF.Exp)

        wq_sb = const.tile([P, KC, DA], bf16, name="wq_sb")
        wk_sb = const.tile([P, KC, DA], bf16, name="wk_sb")
        wv_sb = const.tile([P, KC, KC, P], bf16, name="wv_sb")
        qmn_sb = const.tile([P, nslots], f32, name="qmn_sb")
        dm_sb = const.tile([P, QA], bf16, name="dm_sb")

        kT_sb = persist.tile([P, K], bf16, name="kT_sb")
        qT_sb = persist.tile([P, QA], bf16, name="qT_sb")
        ct_sb = persist.tile([P, KC, K], bf16, name="ct_sb")
        xq_sb = persist.tile([P, KC, QA], bf16, name="xq_sb")
        if vcols:
            xv_sb = persist.tile([P, KC, vcols], bf16, name="xv_sb")
            vosb = persist.tile([P, KC, vcols], bf16, name="vosb")
        xb_sb = persist.tile([P, kmax, D], bf16, name="xb_sb")
        den_sb = persist.tile([P, nslots], f32, name="den_sb")

        es: dict = {}    # (slot, kb128) -> (tile, col_offset)
        dacs: dict = {s: [] for s in range(nslots)}

        # ---- stage emitters ------------------------------------------------

        # kT chunks of 256 keys (+ partial tail): fine granularity so the
        # first matmuls start ~2.5us earlier while ct still streams in
        kcw = [(o, min(256, K - o)) for o in range(0, K, 256)]

        def kT_dma(ci):
            o, w = kcw[ci]
            nc.sync.dma_start(ct_sb[:, :, o:o + w], ct_d[:, :, o:o + w])

        def kT_mm(ci):
            o, w = kcw[ci]
            ps = psl_pool.tile([P, 512], f32, tag="psl", name=f"psk{ci}")
            for kc in range(KC):
                nc.tensor.matmul(ps[:, :w], lhsT=wk_sb[:, kc, :],
                                 rhs=ct_sb[:, kc, o:o + w],
                                 start=(kc == 0), stop=(kc == KC - 1))
            nc.vector.tensor_copy(kT_sb[:, o:o + w], ps[:, :w])

        # qT chunks of 256 query cols (+ partial tail)
        qcw = [(o, min(256, QA - o)) for o in range(0, QA, 256)]

        def qT_dma(ci, eng=None):
            o, w = qcw[ci]
            (eng or nc.sync).dma_start(xq_sb[:, :, o:o + w],
                                       xq_d[:, :, o:o + w])

        def qT_mm(ci):
            o, w = qcw[ci]
            ps = psl_pool.tile([P, 512], f32, tag="psl", name=f"psq{ci}")
            for kc in range(KC):
                nc.tensor.matmul(ps[:, :w], lhsT=wq_sb[:, kc, :],
                                 rhs=xq_sb[:, kc, o:o + w],
                                 start=(kc == 0), stop=(kc == KC - 1))
            nc.vector.tensor_copy(qT_sb[:, o:o + w], ps[:, :w])

        # v-path projection, do-major: per do, one psum group per <=512 cols
        if vcols:
            if vcols > 512:
                h = (vcols + 1) // 2
                vgroups = [(0, h), (h, vcols - h)]
            else:
                vgroups = [(0, vcols)]

        vp_flushed = [0]

        def vp_item(do, gi):
            o, w = vgroups[gi]
            ps = pjp_pool.tile([P, 512], f32, tag="pjp", name=f"psv{do}_{o}")
            for kc in range(KC):
                nc.tensor.matmul(ps[:, :w], lhsT=wv_sb[:, do, kc, :],
                                 rhs=xv_sb[:, kc, o:o + w],
                                 start=(kc == 0), stop=(kc == KC - 1))
            nc.gpsimd.tensor_copy(vosb[:, do, o:o + w], ps[:, :w])
            # flush completed dos in pairs so the output DMA overlaps the
            # remaining compute (fillers pop in do order)
            if gi == len(vgroups) - 1 and (do % 2 == 1 or do == KC - 1):
                d0 = vp_flushed[0]
                nc.scalar.dma_start(outT_r[:, d0:do + 1, QA:],
                                    vosb[:, d0:do + 1, :])
                vp_flushed[0] = do + 1

        # logits+exp for one chunk of one slot. ACT reads the PSUM bank
        # directly; the chunk containing the slot's diagonal 128-block first
        # gets the triangular/pad mask added in-place on DVE.
        def slot_logit_chunk(s, o, w):
            wtot = nkb[s] * P
            psl = psl_pool.tile([P, 512], f32, tag="psl",
                                name=f"psl{s}_{o}")
            nc.tensor.matmul(psl[:, :w], lhsT=qT_sb[:, s * P:(s + 1) * P],
                             rhs=kT_sb[:, o:o + w], start=True, stop=True)
            e = apool.tile([P, 512], bf16, tag="e", name=f"e{s}_{o}",
                           bufs=10)
            dac = apool.tile([P, 1], f32, tag="dac", name=f"dac{s}_{o}",
                             bufs=12)
            if o + w == wtot:  # this chunk ends with the diagonal block
                dc = w - P
                nc.vector.tensor_tensor(
                    out=psl[:, dc:dc + P], in0=psl[:, dc:dc + P],
                    in1=dm_sb[:, s * P:(s + 1) * P], op=ALU.add)
            nc.scalar.activation(e[:, :w], psl[:, :w], AF.Exp,
                                 bias=qmn_sb[:, s:s + 1], scale=1.0,
                                 accum_out=dac[:])
            for ki in range(w // P):
                es[(s, o // P + ki)] = (e, ki * P)
            dacs[s].append(dac)

        # transpose a pair's e blocks; 2 key-blocks share one psT bank/eT
        def pair_transp(pr):
            sl = pairs[pr]
            both_n = nkb[sl[0]] if len(sl) == 2 else 0
            top_n = nkb[sl[-1]]
            eTs = []
            for kh in range((top_n + 1) // 2):
                psT = psT_pool.tile([P, 512], bf16, tag="psT",
                                    name=f"psT{pr}_{kh}")
                eT = epool.tile([P, 512], bf16, tag="eT", name=f"eT{pr}_{kh}")
                runs = []
                for ki in range(2):
                    kb = 2 * kh + ki
                    if kb >= top_n:
                        break
                    base = ki * 256
                    if kb < both_n:
                        t0, o0 = es[(sl[0], kb)]
                        nc.tensor.transpose(psT[:, base:base + P],
                                            t0[:, o0:o0 + P], ident[:])
                        t1, o1 = es[(sl[1], kb)]
                        nc.tensor.transpose(psT[:, base + P:base + 2 * P],
                                            t1[:, o1:o1 + P], ident[:])
                        w = 256
                    else:
                        t1, o1 = es[(sl[-1], kb)]
                        nc.tensor.transpose(psT[:, base:base + P],
                                            t1[:, o1:o1 + P], ident[:])
                        w = P
                    if runs and runs[-1][0] + runs[-1][1] == base:
                        runs[-1] = (runs[-1][0], runs[-1][1] + w)
                    else:
                        runs.append((base, w))
                    eTs.append((eT, base, kb < both_n))
                for bse, w in runs:
                    nc.vector.tensor_copy(eT[:, bse:bse + w], psT[:, bse:bse + w])
            return eTs

        # AX: zT[dm, q] = sum_k x[k, dm] * e[q, k] over the pair's coverage.
        # One item = one psum bank (512 f32) = dpb dm-chunks, a ~0.5-1.5us
        # filler unit.
        def ax_item(pr, zT, eTs, dmh):
            sl = pairs[pr]
            qw = 128 * len(sl)          # 256 for pairs, 128 for lone slot
            dpb = 512 // qw             # dm-chunks packed per psum bank
            top_n = nkb[sl[-1]]
            ps = psax_pool.tile([P, 512], f32, tag="psax",
                                name=f"psax{pr}_{dmh}")
            for sub in range(dpb):
                dmc = dmh * dpb + sub
                base = sub * qw
                for kb in range(top_n):
                    eT, eb, both = eTs[kb]
                    lhsT = xb_sb[:, kb, dmc * P:(dmc + 1) * P]
                    first = (kb == 0 and sub == 0)
                    last = (kb == top_n - 1 and sub == dpb - 1)
                    if both:
                        # blocks are distinct ascending, so the last kb of a
                        # pair always falls in the lone-slot branch below
                        nc.tensor.matmul(ps[:, base:base + P], lhsT=lhsT,
                                         rhs=eT[:, eb:eb + P],
                                         start=first, stop=False)
                        nc.tensor.matmul(
                            ps[:, base + P:base + 2 * P], lhsT=lhsT,
                            rhs=eT[:, eb + P:eb + 2 * P],
                            start=False, stop=False)
                    else:
                        # pair: the surviving blocks belong to the top slot
                        # (second 128 cols); lone slot: the only 128 cols
                        b0 = base + (P if len(sl) == 2 else 0)
                        nc.tensor.matmul(
                            ps[:, b0:b0 + P], lhsT=lhsT,
                            rhs=eT[:, eb:eb + P],
                            start=first, stop=last)
            nc.gpsimd.tensor_copy(zT[:, dmh * dpb:(dmh + 1) * dpb, :], ps[:])

        def pair_den(pr):
            for s in pairs[pr]:
                dl = dacs[s]
                dst = den_sb[:, s:s + 1]
                if len(dl) == 1:
                    nc.vector.tensor_copy(dst, dl[0][:])
                else:
                    nc.vector.tensor_tensor(out=dst, in0=dl[0][:],
                                            in1=dl[1][:], op=ALU.add)
                    for d in dl[2:]:
                        nc.vector.tensor_tensor(out=dst, in0=dst, in1=d[:],
                                                op=ALU.add)

        # attention-side projection item: one dout of one pair's zT.
        # All pairs share one osb staging tile; flushes merge adjacent pairs
        # so every output descriptor is >= 512B (no small-desc penalty).
        osb = persist.tile([P, KC, QA], bf16, name="osb")
        osb_flushed = [0]

        def proj_item(pr, zT, do):
            sl = pairs[pr]
            qw = 128 * len(sl)
            q0 = sl[0] * P
            ps = pjp_pool.tile([P, 512], f32, tag="pjp", name=f"psp{pr}_{do}")
            for dmc in range(KC):
                nc.tensor.matmul(ps[:, :qw], lhsT=wv_sb[:, do, dmc, :],
                                 rhs=zT[:, dmc, :],
                                 start=(dmc == 0), stop=(dmc == KC - 1))
            nc.gpsimd.tensor_copy(osb[:, do, q0:q0 + qw], ps[:, :qw])
            if do == KC - 1:
                f0 = osb_flushed[0]
                unflushed = q0 + qw - f0
                remaining = QA - (q0 + qw)
                if remaining == 0 or (unflushed >= 256 and remaining >= 256):
                    nc.scalar.dma_start(outT_r[:, :, f0:f0 + unflushed],
                                        osb[:, :, f0:f0 + unflushed])
                    osb_flushed[0] = q0 + qw

        # ---- DMA queue -----------------------------------------------------
        # The cost model serializes ALL transfers through one DMA-engines
        # resource (360B/ns, 2x penalty under 512B runs), so parallel queues
        # buy nothing: put every input on the SP HWDGE queue in exact PE
        # consumption order, and outputs on the ACT queue (they must not
        # block later inputs).
        kt_need_pr = [-(-nkb[pairs[pr][-1]] * P // 256)
                      for pr in range(len(pairs))]
        heavy = blocks[0] != 0
        xb_grp = []
        xb_done = 0
        for pr in range(len(pairs)):
            need = nkb[pairs[pr][-1]]
            if need > xb_done:
                xb_grp.append((pr, xb_done, need))
                xb_done = need

        nc.sync.dma_start(wq_sb[:], wq_d)
        qT_dma(0)
        nc.sync.dma_start(wk_sb[:], wk_d)
        # light cores live off v-path fillers from the start: the first xv
        # group + first wv slices lead; heavy cores want ct first
        front_vp = vcols >= 256 and not heavy
        if front_vp:
            nc.sync.dma_start(wv_sb[:, 0], wv_d[:, 0])
            o, w = vgroups[0]
            nc.sync.dma_start(xv_sb[:, :, o:o + w], xv_d[:, :, o:o + w])
            nc.sync.dma_start(wv_sb[:, 1], wv_d[:, 1])
            wv_rest = list(range(2, KC))
        else:
            wv_rest = list(range(KC))
        kd = 0
        for pr in range(len(pairs)):
            while kd < kt_need_pr[pr]:
                kT_dma(kd)
                kd += 1
            if pr == 0:
                nc.sync.dma_start(qmn_sb[:], qmn_d)
                nc.sync.dma_start(dm_sb[:], dm_d)
                if vcols and not front_vp:
                    nc.sync.dma_start(wv_sb[:, 0], wv_d[:, 0])
                    for o, w in vgroups:
                        nc.sync.dma_start(xv_sb[:, :, o:o + w],
                                          xv_d[:, :, o:o + w])
                    nc.sync.dma_start(wv_sb[:, 1], wv_d[:, 1])
                    wv_rest = list(range(2, KC))
                elif front_vp and len(vgroups) > 1:
                    for o, w in vgroups[1:]:
                        nc.sync.dma_start(xv_sb[:, :, o:o + w],
                                          xv_d[:, :, o:o + w])
            if pr + 1 < len(qcw):
                qT_dma(pr + 1)
            # xb group consumed by AX(pr-1) during THIS pair's logits
            for g, a, b_ in xb_grp:
                if g == pr - 1:
                    nc.sync.dma_start(xb_sb[:, a:b_, :], xb_r[:, a:b_, :])
            for _ in range(3 if heavy else 2):
                if wv_rest:
                    do = wv_rest.pop(0)
                    nc.sync.dma_start(wv_sb[:, do], wv_d[:, do])
        for g, a, b_ in xb_grp:
            if g >= len(pairs) - 1:
                nc.sync.dma_start(xb_sb[:, a:b_, :], xb_r[:, a:b_, :])
        while wv_rest:
            do = wv_rest.pop(0)
            nc.sync.dma_start(wv_sb[:, do], wv_d[:, do])

        # ---- PE schedule ---------------------------------------------------
        # Emission order == per-engine execution order. "Fillers" are
        # dependency-free ~1us PE work items (v-path dos, AX of finished
        # pairs, projections) spliced between logits chunks / transposes so
        # the PE never waits on the DVE/ACT exp chain or PSUM turnarounds.
        from collections import deque
        fillers: deque = deque()

        def drain(n):
            while fillers and n > 0:
                fillers.popleft()()
                n -= 1

        # group-major: all dos of xv group 0 first (group 1's DMA lands
        # later on light cores)
        vp_seq = [(do, gi) for gi in range(len(vgroups))
                  for do in range(KC)] if vcols else []
        nfront = min(len(vp_seq), 4) if front_vp else 0
        for it in vp_seq[:nfront]:
            fillers.append(lambda it=it: vp_item(*it))
        vp_rest = deque(vp_seq[nfront:])

        # front: qT0 (its inputs lead the queue), then kT chunks chasing
        # the ct DMA stream with fillers between
        qT_mm(0)
        kt_done = 0
        while kt_done < kt_need_pr[0]:
            kT_mm(kt_done)
            kt_done += 1
            drain(1)
        drain(max(0, len(fillers) - 1))

        zTs = {}
        for pr in range(len(pairs)):
            if pr + 1 < len(qcw):
                qT_mm(pr + 1)
            while kt_done < kt_need_pr[pr]:
                kT_mm(kt_done)
                kt_done += 1
            # logits chunks with a filler after every second chunk
            nch = 0
            for s in pairs[pr]:
                wtot = nkb[s] * P
                for o in range(0, wtot, 512):
                    slot_logit_chunk(s, o, min(512, wtot - o))
                    nch += 1
                    if nch % 2 == 0:
                        drain(1)
            drain(1)
            eTs = pair_transp(pr)
            pair_den(pr)
            # queue this pair's AX; its completion queues the projections
            sl = pairs[pr]
            qw = 128 * len(sl)
            dpb = 512 // qw
            zT = apool.tile([P, KC, qw], bf16, tag="zT", name=f"zT{pr}",
                            bufs=4)
            zTs[pr] = zT

            def make_ax(pr=pr, zT=zT, eTs=eTs, dpb=dpb):
                def run(dmh):
                    ax_item(pr, zT, eTs, dmh)
                    if dmh == KC // dpb - 1:
                        for do in range(KC):
                            fillers.append(
                                lambda do=do: proj_item(pr, zT, do))
                return run
            ax_run = make_ax()
            # interleave v-path items between AX items: AX may briefly wait
            # on its xb group, and the v-path keeps the PE fed meanwhile
            for dmh in range(KC // dpb):
                if vp_rest and dmh % 2 == 0:
                    fillers.append(
                        lambda it=vp_rest.popleft(): vp_item(*it))
                fillers.append(lambda dmh=dmh, ax_run=ax_run: ax_run(dmh))

        nc.scalar.dma_start(den_d, den_sb[:])
        # tail: drain AX/proj fillers and leftover v-path items
        while fillers or vp_rest:
            if fillers:
                fillers.popleft()()
            elif vp_rest:
                vp_item(*vp_rest.popleft())

    nc.compile()
    return nc


def _get_programs(mask_np):
    key = mask_np.tobytes()
    with _BUILD_LOCK:
        if _CACHE.get("key") != key:
            cfgs = _plan(mask_np)
            progs = {}
            for cfg in cfgs:
                sig = (cfg["kmax"], cfg["blocks"], cfg["vcols"])
                if sig not in progs:
                    progs[sig] = _build(*sig)
            _CACHE.update(key=key, cfgs=cfgs, progs=progs)
        return _CACHE["cfgs"], _CACHE["progs"]


def _get_ncs():
    return tuple(_CACHE["progs"].values())


# --------------------------------------------------------------------------
# host side
# --------------------------------------------------------------------------

def make_in_maps(x, cross, Wq, Wk, Wv, mask, cfgs):
    bf = ml_dtypes.bfloat16
    x = np.asarray(x, dtype=np.float32)
    cross = np.asarray(cross, dtype=np.float32)
    mask_np = np.asarray(mask)
    scale = 1.0 / math.sqrt(DA)
    wq_h = np.ascontiguousarray(
        (np.asarray(Wq, np.float32) * scale).T.reshape(KC, P, DA)
        .transpose(1, 0, 2)).astype(bf)
    wk_h = np.ascontiguousarray(
        np.asarray(Wk, np.float32).T.reshape(KC, P, DA)
        .transpose(1, 0, 2)).astype(bf)
    # wv do-major: wv_h[p, do, kc, j] = Wv[do*128+j, kc*128+p]
    wv_h = np.ascontiguousarray(
        np.asarray(Wv, np.float32).T.reshape(KC, P, KC, P)
        .transpose(1, 2, 0, 3)).astype(bf)

    # per-batch compacted order: unmasked asc, then masked asc
    orders, caps = [], []
    for b in range(B):
        m = mask_np[b].astype(bool)
        un = np.flatnonzero(m)
        ma = np.flatnonzero(~m)
        cap = cfgs[2 * b]["nblk"] * P
        orders.append(np.concatenate([un, ma[:cap - len(un)]]))
        caps.append(cap)
    # global leftover v-pool: (batch, q) pairs
    vpool = []
    for b in range(B):
        m = mask_np[b].astype(bool)
        ma = np.flatnonzero(~m)
        for q in ma[caps[b] - int(m.sum()):]:
            vpool.append((b, int(q)))
    assert len(vpool) == sum(c["vcols"] for c in cfgs)

    in_maps, metas = [], []
    vo = 0
    for c, cfg in enumerate(cfgs):
        b = cfg["batch"]
        blocks = cfg["blocks"]
        kmax, nb = cfg["kmax"], cfg["nb"]
        order = orders[b]
        K = kmax * P
        qpos = np.concatenate([order[g * P:(g + 1) * P] for g in blocks])
        vlist = vpool[vo:vo + cfg["vcols"]]
        vo += cfg["vcols"]

        ct_h = np.ascontiguousarray(
            cross[b][order[:K]].T.reshape(KC, P, K)
            .transpose(1, 0, 2)).astype(bf)
        xb_h = np.ascontiguousarray(x[b][order[:K]]).astype(bf)
        xq_h = np.ascontiguousarray(
            x[b][qpos].T.reshape(KC, P, len(qpos))
            .transpose(1, 0, 2)).astype(bf)

        qmn_h = np.zeros((P, len(blocks)), np.float32)
        dm_h = np.zeros((P, len(blocks) * P), np.float32)
        rows = np.arange(P)
        for s, g in enumerate(blocks):
            padded = g * P + rows >= nb          # [P] bool
            qmn_h[padded, s] = -BIG
            tri = np.where(rows[None, :] <= rows[:, None], 0.0, -BIG)
            pad_row = np.where(rows[None, :] == rows[:, None], BIG, -BIG)
            dm_h[:, s * P:(s + 1) * P] = np.where(padded[:, None],
                                                  pad_row, tri)
        im = {"ct": ct_h, "xq": xq_h, "wq": wq_h, "wk": wk_h,
              "wv": wv_h, "xb": xb_h, "qmn": qmn_h,
              "dmask": dm_h.astype(bf)}
        if cfg["vcols"]:
            xv_rows = np.stack([x[bb, qq] for bb, qq in vlist])
            im["xv"] = np.ascontiguousarray(
                xv_rows.T.reshape(KC, P, len(vlist))
                .transpose(1, 0, 2)).astype(bf)
        in_maps.append(im)
        metas.append({"batch": b, "qpos": qpos, "vlist": vlist,
                      "nslots": len(blocks)})
    return in_maps, metas


def kernel(x, cross, Wq, Wk, Wv, mask):
    from concourse import bass_utils

    mask_np = np.asarray(mask)
    cfgs, progs = _get_programs(mask_np)
    in_maps, metas = make_in_maps(x, cross, Wq, Wk, Wv, mask, cfgs)

    # group cores by program
    groups: dict = {}
    for c, cfg in enumerate(cfgs):
        sig = (cfg["kmax"], cfg["blocks"], cfg["vcols"])
        groups.setdefault(sig, []).append(c)

    results = {}
    for sig, cores in groups.items():
        res = bass_utils.run_bass_kernel_spmd(
            progs[sig], [in_maps[c] for c in cores], core_ids=cores)
        for i, c in enumerate(cores):
            results[c] = res.results[i]

    out = np.empty((B, S, D), np.float32)
    for c, meta in enumerate(metas):
        r = results[c]
        o = r["outT"].astype(np.float32).T  # [Z, 1024]
        qa = len(meta["qpos"])
        denf = r["den"].T.reshape(-1).astype(np.float32)  # [QA] slot-major
        out[meta["batch"], meta["qpos"]] = o[:qa] / denf[:, None]
        for i, (bb, qq) in enumerate(meta["vlist"]):
            out[bb, qq] = o[qa + i]
    return out


# revision 32
# speedup vs baseline: 1.1016x; 1.0698x over previous
"""Trainium2 Bass kernel v4 for nn_Attention_42288247996512.

reference:
  q = x @ Wq.T; k = cross @ Wk.T; v = x @ Wv.T
  logits = q @ k.T  (causal; padding m_q*m_k, diag always kept)
  out = softmax(logits / sqrt(128)) @ v

v4 = v3 (mask compaction) + fp8 DoubleRow on every big matmul.

Mask compaction (v3): keys and queries share the SAME padding mask, so
attention over the ~50% unmasked positions is PURE causal attention in
mask-compacted coordinates; masked queries reduce to out[q]=x[q]@Wv.T,
a batch-independent "v-path" Wv projection freely assignable to any
core for balance. 2 cores per batch (heavy = top contiguous compacted
blocks, light = bottom prefix) + water-filled v-path quotas equalize
all 8 cores.

fp8 (v4): the PE runs fp8e4 matmuls in DoubleRow mode at 0.5
cycles/row, contracting TWO stacked K-tiles per instruction. Used as:
  * kT/qT at 2x: stationary Wk/Wq (x16, single fp8) broadcast across
    both K-tiles; moving cross/x as (hi,lo) fp8 pairs -> operand error
    only from the small weights (~0.5% on the output).
  * AX at 2x: stationary x as real (hi,lo) pairs; moving e^T single
    fp8 broadcast (softmax renormalization damps e-quantization to
    ~0.3%).
  * proj at 1.33x: Wv x64 split host-side into W0=fp8(64Wv) and
    R=fp8(64Wv-W0); z split on-chip into (zh,zl) fp8. out = W0@(zh+zl)
    [one DoubleRow per dm-chunk] + R@zh [one DoubleRow per dm-chunk
    PAIR], dropping only R@zl ~ 0.2%.
Logits and transposes stay bf16 (q/k precision is softmax-critical).
The 1/sqrt(128) and the x16/x64 weight scalings fold into the ACT exp
scale and the host-side divide; exp also gets a constant -3 bias so
the unnormalized e/z stay inside fp8e4 range. Measured end-to-end
rel err ~7.8e-3 (limit 2e-2).

All inputs ride the SP HWDGE queue in exact consumption order (the
cost model serializes all DMA transfers through one engine pool);
outputs ride ACT. Copies are pinned: Pool takes psum drains, DVE takes
eT/kT/qT + the z hi/lo splits.
"""
import math
import threading

import ml_dtypes
import numpy as np

B, S, D, DA = 4, 2048, 1024, 128
P = 128
KC = D // P  # 8 contraction chunks
NCORES = 8
BIG = 8388608.0   # 2^23: decisive after the 1/(256*sqrt(128)) exp scale
SW = 64.0         # Wv host prescale
SQ = 16.0         # Wq/Wk host prescale
CBIAS = 3.0       # constant exp bias keeps e', z in fp8e4 range
ESCALE = 1.0 / (SQ * SQ * math.sqrt(DA))
PCOL = 48         # proj cycles per output column (fp8 3-term scheme)

_BUILD_LOCK = threading.Lock()
_CACHE: dict = {}


# --------------------------------------------------------------------------
# planning: per-batch split + v-path quota water-fill
# --------------------------------------------------------------------------

def _core_cycles(kmax, nslots, wsum):
    # kT + qT (fp8 2x) + logits + transp (bf16) + AX (fp8 2x), in PE cycles
    return 512 * (kmax + nslots) + 768 * wsum


def _plan(mask_np):
    """Returns per-core cfg list. Core 2b = heavy(batch b), 2b+1 = light."""
    nbs = mask_np.astype(np.int64).sum(1)  # unmasked per batch
    nblks = [max(2, -(-int(nb) // 128)) for nb in nbs]

    best = None
    import itertools
    for js in itertools.product(*[range(1, nblk) for nblk in nblks]):
        cores = []  # (A_cycles, nslots)
        for nblk, j in zip(nblks, js):
            wh = nblk * (nblk + 1) // 2 - j * (j + 1) // 2
            cores.append((_core_cycles(nblk, nblk - j, wh), nblk - j))
            cores.append((_core_cycles(j, j, j * (j + 1) // 2), j))
        zmin = [128 * n for _, n in cores]
        free = list(range(8))
        fixed_z = 0.0
        while True:
            t = (sum(cores[c][0] for c in free) + PCOL * (8192 - fixed_z)) \
                / len(free)
            newly = [c for c in free
                     if cores[c][0] + PCOL * zmin[c] > t + 1e-9]
            if not newly:
                break
            for c in newly:
                free.remove(c)
                fixed_z += zmin[c]
        obj = max(t, max((cores[c][0] + PCOL * zmin[c]
                          for c in range(8) if c not in free), default=0))
        if best is None or obj < best[0]:
            best = (obj, js, t, list(free))
    _, js, t, free = best

    cfgs = []
    zs = []
    for b, (nblk, j) in enumerate(zip(nblks, js)):
        for hv, blocks in ((0, tuple(range(j, nblk))),
                           (1, tuple(range(j)))):
            c = 2 * b + hv
            kmax = max(blocks) + 1
            w = sum(g + 1 for g in blocks)
            a = _core_cycles(kmax, len(blocks), w)
            z = (t - a) / PCOL if c in free else 128.0 * len(blocks)
            zs.append(max(z, 128.0 * len(blocks)))
            cfgs.append({"batch": b, "blocks": blocks, "kmax": kmax,
                         "nb": int(nbs[b]), "nblk": nblk, "acyc": a})
    zi = [int(z) for z in zs]
    cls = {}
    for c, cfg in enumerate(cfgs):
        cls.setdefault((cfg["nblk"], cfg["blocks"]), []).append(c)
    for members in cls.values():
        zm = min(zi[c] for c in members)
        for c in members:
            zi[c] = zm
    rem = 8192 - sum(zi)
    order = sorted(cls.values(), key=len)
    progress = True
    while rem > 0 and progress:
        progress = False
        for members in order:
            if rem >= len(members):
                for c in members:
                    zi[c] += 1
                rem -= len(members)
                progress = True
    if rem > 0:
        zi[order[0][0]] += rem
    for c, cfg in enumerate(cfgs):
        cfg["vcols"] = zi[c] - 128 * len(cfg["blocks"])
        assert cfg["vcols"] >= 0
    assert sum(zi) == 8192
    return cfgs


# --------------------------------------------------------------------------
# kernel builder (one program per distinct (kmax, blocks, vcols))
# --------------------------------------------------------------------------

def _build(kmax, blocks, vcols):
    from contextlib import ExitStack

    import concourse.mybir as mybir
    import concourse.tile as tile
    from concourse import bacc
    from concourse.masks import make_identity

    dt = mybir.dt
    f32 = dt.float32
    bf16 = dt.bfloat16
    f8 = dt.float8e4
    AF = mybir.ActivationFunctionType
    ALU = mybir.AluOpType
    DR = mybir.MatmulPerfMode.DoubleRow

    nslots = len(blocks)
    K = kmax * P          # keys held on this core
    QA = nslots * P       # attention query columns
    Z = QA + vcols        # total output columns
    pairs = [tuple(range(i, min(i + 2, nslots))) for i in range(0, nslots, 2)]
    nkb = [g + 1 for g in blocks]
    KCH = -(-K // 256)    # 256-key hi/lo-interleaved chunks
    QCH = -(-QA // 256)

    nc = bacc.Bacc("TRN2", target_bir_lowering=False, debug=False)

    # hi/lo pairs interleaved per 256-col chunk: slice [:, kc, ci] is a
    # [P, 2, 256] DoubleRow moving operand with >=512B DMA rows
    ct_d = nc.dram_tensor("ct", [P, KC, KCH, 2, 256], f8,
                          kind="ExternalInput").ap()
    xq_d = nc.dram_tensor("xq", [P, KC, QCH, 2, 256], f8,
                          kind="ExternalInput").ap()
    if vcols:
        xv_d = nc.dram_tensor("xv", [P, KC, 2, vcols], f8,
                              kind="ExternalInput").ap()
    wq_d = nc.dram_tensor("wq", [P, KC, DA], f8, kind="ExternalInput").ap()
    wk_d = nc.dram_tensor("wk", [P, KC, DA], f8, kind="ExternalInput").ap()
    wv0_d = nc.dram_tensor("wv0", [P, KC, KC, P], f8,
                           kind="ExternalInput").ap()   # [p, do, kc, 128]
    wvr_d = nc.dram_tensor("wvr", [P, KC, KC // 2, 2, P], f8,
                           kind="ExternalInput").ap()   # [p, do, kcp, 2, 128]
    xhl_d = nc.dram_tensor("xhl", [P, kmax, 2, D], f8,
                           kind="ExternalInput").ap()   # [p, block, hi/lo, d]
    qmn_d = nc.dram_tensor("qmn", [P, nslots], f32, kind="ExternalInput").ap()
    dm_d = nc.dram_tensor("dmask", [P, QA], bf16, kind="ExternalInput").ap()

    outT = nc.dram_tensor("outT", [D, Z], bf16, kind="ExternalOutput").ap()
    den_d = nc.dram_tensor("den", [P, nslots], f32, kind="ExternalOutput").ap()

    outT_r = outT.rearrange("(do p) q -> p do q", p=P)

    with tile.TileContext(nc) as tc, ExitStack() as ctx:
        const = ctx.enter_context(tc.tile_pool(name="const", bufs=1))
        persist = ctx.enter_context(tc.tile_pool(name="persist", bufs=1))
        apool = ctx.enter_context(tc.tile_pool(name="apool", bufs=4))
        epool = ctx.enter_context(tc.tile_pool(name="epool", bufs=16))
        psl_pool = ctx.enter_context(tc.tile_pool(name="psl", bufs=2,
                                                  space="PSUM"))
        psT_pool = ctx.enter_context(tc.tile_pool(name="psT", bufs=2,
                                                  space="PSUM"))
        psax_pool = ctx.enter_context(tc.tile_pool(name="psax", bufs=2,
                                                   space="PSUM"))
        pjp_pool = ctx.enter_context(tc.tile_pool(name="pjp", bufs=2,
                                                  space="PSUM"))

        ident_f32 = const.tile([P, P], f32, name="ident_f32")
        make_identity(nc, ident_f32)
        ident = const.tile([P, P], bf16, name="ident")
        nc.vector.tensor_copy(ident[:], ident_f32[:])
        # touch Exp once so ACT loads its table during the front DMA window
        scratch = const.tile([1, 1], f32, name="scratch")
        nc.scalar.activation(scratch[:], ident_f32[0:1, 0:1], AF.Exp)

        wq_sb = const.tile([P, KC, DA], f8, name="wq_sb")
        wk_sb = const.tile([P, KC, DA], f8, name="wk_sb")
        wv0_sb = const.tile([P, KC, KC, P], f8, name="wv0_sb")
        wvr_sb = const.tile([P, KC, KC // 2, 2, P], f8, name="wvr_sb")
        qmn_sb = const.tile([P, nslots], f32, name="qmn_sb")
        dm_sb = const.tile([P, QA], bf16, name="dm_sb")

        kT_sb = persist.tile([P, K], bf16, name="kT_sb")
        qT_sb = persist.tile([P, QA], bf16, name="qT_sb")
        ct_sb = persist.tile([P, KC, KCH, 2, 256], f8, name="ct_sb")
        xq_sb = persist.tile([P, KC, QCH, 2, 256], f8, name="xq_sb")
        if vcols:
            xv_sb = persist.tile([P, KC, 2, vcols], f8, name="xv_sb")
            vosb = persist.tile([P, KC, vcols], bf16, name="vosb")
        xhl_sb = persist.tile([P, kmax, 2, D], f8, name="xhl_sb")
        den_sb = persist.tile([P, nslots], f32, name="den_sb")

        es: dict = {}    # (slot, kb128) -> (tile, col_offset)
        dacs: dict = {s: [] for s in range(nslots)}

        # ---- stage emitters ------------------------------------------------

        kcw = [(ci, min(256, K - ci * 256)) for ci in range(KCH)]
        qcw = [(ci, min(256, QA - ci * 256)) for ci in range(QCH)]

        def kT_dma(ci):
            nc.sync.dma_start(ct_sb[:, :, ci], ct_d[:, :, ci])

        def kT_mm(ci):
            _, w = kcw[ci]
            ps = psl_pool.tile([P, 512], f32, tag="psl", name=f"psk{ci}")
            for kc in range(KC):
                nc.tensor.matmul(
                    ps[:, :w],
                    lhsT=wk_sb[:, kc, :].unsqueeze(1).broadcast_to([P, 2, DA]),
                    rhs=ct_sb[:, kc, ci, :, :w], perf_mode=DR,
                    start=(kc == 0), stop=(kc == KC - 1))
            nc.vector.tensor_copy(kT_sb[:, ci * 256:ci * 256 + w], ps[:, :w])

        def qT_dma(ci, eng=None):
            (eng or nc.sync).dma_start(xq_sb[:, :, ci], xq_d[:, :, ci])

        def qT_mm(ci):
            _, w = qcw[ci]
            ps = psl_pool.tile([P, 512], f32, tag="psl", name=f"psq{ci}")
            for kc in range(KC):
                nc.tensor.matmul(
                    ps[:, :w],
                    lhsT=wq_sb[:, kc, :].unsqueeze(1).broadcast_to([P, 2, DA]),
                    rhs=xq_sb[:, kc, ci, :, :w], perf_mode=DR,
                    start=(kc == 0), stop=(kc == KC - 1))
            nc.vector.tensor_copy(qT_sb[:, ci * 256:ci * 256 + w], ps[:, :w])

        # fp8 3-term projection core: W0@(zh+zl) + R@zh; rhs_a(c) yields the
        # [P, 2, w] (hi,lo) view of dm-chunk c, rhs_b(cp) the (hi_c, hi_c+1)
        # chunk-pair view
        def proj_mms(ps, w, do, rhs_a, rhs_b):
            for c in range(KC):
                nc.tensor.matmul(
                    ps[:, :w],
                    lhsT=wv0_sb[:, do, c, :].unsqueeze(1)
                    .broadcast_to([P, 2, P]),
                    rhs=rhs_a(c), perf_mode=DR,
                    start=(c == 0), stop=False)
            for cp in range(KC // 2):
                nc.tensor.matmul(
                    ps[:, :w], lhsT=wvr_sb[:, do, cp], rhs=rhs_b(cp),
                    perf_mode=DR, start=False, stop=(cp == KC // 2 - 1))

        # v-path projection, do-major
        if vcols:
            if vcols > 512:
                h = (vcols + 1) // 2
                vgroups = [(0, h), (h, vcols - h)]
            else:
                vgroups = [(0, vcols)]
        vp_flushed = [0]

        def vp_item(do, gi):
            o, w = vgroups[gi]
            ps = pjp_pool.tile([P, 512], f32, tag="pjp", name=f"psv{do}_{o}")
            proj_mms(ps, w, do,
                     lambda c: xv_sb[:, c, :, o:o + w],
                     lambda cp: xv_sb[:, 2 * cp:2 * cp + 2, 0, o:o + w])
            nc.gpsimd.tensor_copy(vosb[:, do, o:o + w], ps[:, :w])
            if gi == len(vgroups) - 1 and (do % 2 == 1 or do == KC - 1):
                d0 = vp_flushed[0]
                nc.scalar.dma_start(outT_r[:, d0:do + 1, QA:],
                                    vosb[:, d0:do + 1, :])
                vp_flushed[0] = do + 1

        # logits+exp for one chunk of one slot (bf16 matmul; ACT reads the
        # PSUM bank directly; the diagonal block's triangular/pad mask is
        # added in-place on DVE first)
        def slot_logit_chunk(s, o, w):
            wtot = nkb[s] * P
            psl = psl_pool.tile([P, 512], f32, tag="psl",
                                name=f"psl{s}_{o}")
            nc.tensor.matmul(psl[:, :w], lhsT=qT_sb[:, s * P:(s + 1) * P],
                             rhs=kT_sb[:, o:o + w], start=True, stop=True)
            e = apool.tile([P, 512], bf16, tag="e", name=f"e{s}_{o}",
                           bufs=10)
            dac = apool.tile([P, 1], f32, tag="dac", name=f"dac{s}_{o}",
                             bufs=12)
            if o + w == wtot:  # chunk ends with the diagonal block
                dc = w - P
                nc.vector.tensor_tensor(
                    out=psl[:, dc:dc + P], in0=psl[:, dc:dc + P],
                    in1=dm_sb[:, s * P:(s + 1) * P], op=ALU.add)
            nc.scalar.activation(e[:, :w], psl[:, :w], AF.Exp,
                                 bias=qmn_sb[:, s:s + 1], scale=ESCALE,
                                 accum_out=dac[:])
            for ki in range(w // P):
                es[(s, o // P + ki)] = (e, ki * P)
            dacs[s].append(dac)

        # transpose a pair's e blocks (bf16); eT copies cast to fp8
        def pair_transp(pr):
            sl = pairs[pr]
            both_n = nkb[sl[0]] if len(sl) == 2 else 0
            top_n = nkb[sl[-1]]
            eTs = []
            for kh in range((top_n + 1) // 2):
                psT = psT_pool.tile([P, 512], bf16, tag="psT",
                                    name=f"psT{pr}_{kh}")
                eT = epool.tile([P, 512], f8, tag="eT", name=f"eT{pr}_{kh}")
                runs = []
                for ki in range(2):
                    kb = 2 * kh + ki
                    if kb >= top_n:
                        break
                    base = ki * 256
                    if kb < both_n:
                        t0, o0 = es[(sl[0], kb)]
                        nc.tensor.transpose(psT[:, base:base + P],
                                            t0[:, o0:o0 + P], ident[:])
                        t1, o1 = es[(sl[1], kb)]
                        nc.tensor.transpose(psT[:, base + P:base + 2 * P],
                                            t1[:, o1:o1 + P], ident[:])
                        w = 256
                    else:
                        t1, o1 = es[(sl[-1], kb)]
                        nc.tensor.transpose(psT[:, base:base + P],
                                            t1[:, o1:o1 + P], ident[:])
                        w = P
                    if runs and runs[-1][0] + runs[-1][1] == base:
                        runs[-1] = (runs[-1][0], runs[-1][1] + w)
                    else:
                        runs.append((base, w))
                    eTs.append((eT, base, kb < both_n))
                for bse, w in runs:
                    nc.vector.tensor_copy(eT[:, bse:bse + w],
                                          psT[:, bse:bse + w])
            return eTs

        # AX: zT[dm, q] = sum_k x[k, dm]*e[q, k]; x as (hi,lo) stationary
        # pair, e^T fp8 broadcast moving. One item = one psum bank; its
        # drain also performs the zh/zl split for the fp8 projection.
        def ax_item(pr, zhl, eTs, dmh):
            sl = pairs[pr]
            qw = 128 * len(sl)
            dpb = 512 // qw
            top_n = nkb[sl[-1]]
            ps = psax_pool.tile([P, 512], f32, tag="psax",
                                name=f"psax{pr}_{dmh}")
            for sub in range(dpb):
                dmc = dmh * dpb + sub
                base = sub * qw
                for kb in range(top_n):
                    eT, eb, both = eTs[kb]
                    lhsT = xhl_sb[:, kb, :, dmc * P:(dmc + 1) * P]
                    first = (kb == 0 and sub == 0)
                    last = (kb == top_n - 1 and sub == dpb - 1)
                    if both:
                        nc.tensor.matmul(
                            ps[:, base:base + P], lhsT=lhsT,
                            rhs=eT[:, eb:eb + P].unsqueeze(1)
                            .broadcast_to([P, 2, P]),
                            perf_mode=DR, start=first, stop=False)
                        nc.tensor.matmul(
                            ps[:, base + P:base + 2 * P], lhsT=lhsT,
                            rhs=eT[:, eb + P:eb + 2 * P].unsqueeze(1)
                            .broadcast_to([P, 2, P]),
                            perf_mode=DR, start=False, stop=False)
                    else:
                        b0 = base + (P if len(sl) == 2 else 0)
                        nc.tensor.matmul(
                            ps[:, b0:b0 + P], lhsT=lhsT,
                            rhs=eT[:, eb:eb + P].unsqueeze(1)
                            .broadcast_to([P, 2, P]),
                            perf_mode=DR, start=first, stop=last)
            # drain with hi/lo split: zh = fp8(z); zl = fp8(z - zh)
            for sub in range(dpb):
                dmc = dmh * dpb + sub
                base = sub * qw
                nc.gpsimd.tensor_copy(zhl[:, dmc, 0, :],
                                      ps[:, base:base + qw])
                nc.vector.tensor_tensor(out=zhl[:, dmc, 1, :],
                                        in0=ps[:, base:base + qw],
                                        in1=zhl[:, dmc, 0, :],
                                        op=ALU.subtract)

        def pair_den(pr):
            for s in pairs[pr]:
                dl = dacs[s]
                dst = den_sb[:, s:s + 1]
                if len(dl) == 1:
                    nc.vector.tensor_copy(dst, dl[0][:])
                else:
                    nc.vector.tensor_tensor(out=dst, in0=dl[0][:],
                                            in1=dl[1][:], op=ALU.add)
                    for d in dl[2:]:
                        nc.vector.tensor_tensor(out=dst, in0=dst, in1=d[:],
                                                op=ALU.add)

        # attention-side projection item: one dout of one pair's zhl.
        # All pairs share one osb staging tile; flushes merge adjacent pairs
        # so every output descriptor is >= 512B.
        osb = persist.tile([P, KC, QA], bf16, name="osb")
        osb_flushed = [0]

        def proj_item(pr, zhl, do):
            sl = pairs[pr]
            qw = 128 * len(sl)
            q0 = sl[0] * P
            ps = pjp_pool.tile([P, 512], f32, tag="pjp", name=f"psp{pr}_{do}")
            proj_mms(ps, qw, do,
                     lambda c: zhl[:, c, :, :],
                     lambda cp: zhl[:, 2 * cp:2 * cp + 2, 0, :])
            nc.gpsimd.tensor_copy(osb[:, do, q0:q0 + qw], ps[:, :qw])
            if do == KC - 1:
                f0 = osb_flushed[0]
                unflushed = q0 + qw - f0
                remaining = QA - (q0 + qw)
                if remaining == 0 or (unflushed >= 256 and remaining >= 256):
                    nc.scalar.dma_start(outT_r[:, :, f0:f0 + unflushed],
                                        osb[:, :, f0:f0 + unflushed])
                    osb_flushed[0] = q0 + qw

        # ---- DMA queue -----------------------------------------------------
        kt_need_pr = [-(-nkb[pairs[pr][-1]] * P // 256)
                      for pr in range(len(pairs))]
        heavy = blocks[0] != 0
        xb_grp = []
        xb_done = 0
        for pr in range(len(pairs)):
            need = nkb[pairs[pr][-1]]
            if need > xb_done:
                xb_grp.append((pr, xb_done, need))
                xb_done = need

        nc.sync.dma_start(wq_sb[:], wq_d)
        qT_dma(0)
        nc.sync.dma_start(wk_sb[:], wk_d)
        front_vp = vcols >= 256 and not heavy
        if front_vp:
            nc.sync.dma_start(wv0_sb[:, 0], wv0_d[:, 0])
            nc.sync.dma_start(wvr_sb[:, 0], wvr_d[:, 0])
            nc.sync.dma_start(xv_sb[:], xv_d)
            nc.sync.dma_start(wv0_sb[:, 1], wv0_d[:, 1])
            nc.sync.dma_start(wvr_sb[:, 1], wvr_d[:, 1])
            wv_rest = list(range(2, KC))
        else:
            wv_rest = list(range(KC))
        kd = 0
        for pr in range(len(pairs)):
            while kd < kt_need_pr[pr]:
                kT_dma(kd)
                kd += 1
            if pr == 0:
                nc.sync.dma_start(qmn_sb[:], qmn_d)
                nc.sync.dma_start(dm_sb[:], dm_d)
                if vcols and not front_vp:
                    nc.sync.dma_start(wv0_sb[:, 0], wv0_d[:, 0])
                    nc.sync.dma_start(wvr_sb[:, 0], wvr_d[:, 0])
                    nc.sync.dma_start(xv_sb[:], xv_d)
                    nc.sync.dma_start(wv0_sb[:, 1], wv0_d[:, 1])
                    nc.sync.dma_start(wvr_sb[:, 1], wvr_d[:, 1])
                    wv_rest = list(range(2, KC))
            if pr + 1 < len(qcw):
                qT_dma(pr + 1)
            # xhl group consumed by AX(pr-1) during THIS pair's logits
            for g, a, b_ in xb_grp:
                if g == pr - 1:
                    nc.sync.dma_start(xhl_sb[:, a:b_], xhl_d[:, a:b_])
            for _ in range(3 if heavy else 2):
                if wv_rest:
                    do = wv_rest.pop(0)
                    nc.sync.dma_start(wv0_sb[:, do], wv0_d[:, do])
                    nc.sync.dma_start(wvr_sb[:, do], wvr_d[:, do])
        for g, a, b_ in xb_grp:
            if g >= len(pairs) - 1:
                nc.sync.dma_start(xhl_sb[:, a:b_], xhl_d[:, a:b_])
        while wv_rest:
            do = wv_rest.pop(0)
            nc.sync.dma_start(wv0_sb[:, do], wv0_d[:, do])
            nc.sync.dma_start(wvr_sb[:, do], wvr_d[:, do])

        # ---- PE schedule ---------------------------------------------------
        from collections import deque
        fillers: deque = deque()

        def drain(n):
            while fillers and n > 0:
                fillers.popleft()()
                n -= 1

        vp_seq = [(do, gi) for gi in range(len(vgroups))
                  for do in range(KC)] if vcols else []
        nfront = min(len(vp_seq), 4) if front_vp else 0
        for it in vp_seq[:nfront]:
            fillers.append(lambda it=it: vp_item(*it))
        vp_rest = deque(vp_seq[nfront:])

        qT_mm(0)
        kt_done = 0
        while kt_done < kt_need_pr[0]:
            kT_mm(kt_done)
            kt_done += 1
            drain(1)
        drain(max(0, len(fillers) - 1))

        zhls = {}
        for pr in range(len(pairs)):
            if pr + 1 < len(qcw):
                qT_mm(pr + 1)
            while kt_done < kt_need_pr[pr]:
                kT_mm(kt_done)
                kt_done += 1
            nch = 0
            for s in pairs[pr]:
                wtot = nkb[s] * P
                for o in range(0, wtot, 512):
                    slot_logit_chunk(s, o, min(512, wtot - o))
                    nch += 1
                    if nch % 2 == 0:
                        drain(1)
            drain(1)
            eTs = pair_transp(pr)
            pair_den(pr)
            sl = pairs[pr]
            qw = 128 * len(sl)
            dpb = 512 // qw
            zhl = apool.tile([P, KC, 2, qw], f8, tag="zhl", name=f"zhl{pr}",
                             bufs=4)
            zhls[pr] = zhl

            def make_ax(pr=pr, zhl=zhl, eTs=eTs, dpb=dpb):
                def run(dmh):
                    ax_item(pr, zhl, eTs, dmh)
                    if dmh == KC // dpb - 1:
                        for do in range(KC):
                            fillers.append(
                                lambda do=do: proj_item(pr, zhl, do))
                return run
            ax_run = make_ax()
            for dmh in range(KC // dpb):
                if vp_rest and dmh % 2 == 0:
                    fillers.append(
                        lambda it=vp_rest.popleft(): vp_item(*it))
                fillers.append(lambda dmh=dmh, ax_run=ax_run: ax_run(dmh))

        nc.scalar.dma_start(den_d, den_sb[:])
        while fillers or vp_rest:
            if fillers:
                fillers.popleft()()
            elif vp_rest:
                vp_item(*vp_rest.popleft())

    nc.compile()
    return nc


def _get_programs(mask_np):
    key = mask_np.tobytes()
    with _BUILD_LOCK:
        if _CACHE.get("key") != key:
            cfgs = _plan(mask_np)
            progs = {}
            for cfg in cfgs:
                sig = (cfg["kmax"], cfg["blocks"], cfg["vcols"])
                if sig not in progs:
                    progs[sig] = _build(*sig)
            _CACHE.update(key=key, cfgs=cfgs, progs=progs)
        return _CACHE["cfgs"], _CACHE["progs"]


def _get_ncs():
    return tuple(_CACHE["progs"].values())


# --------------------------------------------------------------------------
# host side
# --------------------------------------------------------------------------

def _hl(a):
    """fp8e4 (hi, lo) pair of a float32 array."""
    f8 = ml_dtypes.float8_e4m3
    hi = a.astype(f8)
    lo = (a - hi.astype(np.float32)).astype(f8)
    return hi, lo


def _chunk_hl(mat, width):
    """[D, N] -> [P, KC, NCH, 2, width] fp8 hi/lo interleaved per chunk."""
    D_, N = mat.shape
    nch = -(-N // width)
    pad = nch * width - N
    if pad:
        mat = np.concatenate([mat, np.zeros((D_, pad), mat.dtype)], axis=1)
    hi, lo = _hl(mat)
    st = np.stack([hi, lo], axis=1)               # [D, 2, nch*width]
    st = st.reshape(KC, P, 2, nch, width)
    return np.ascontiguousarray(st.transpose(1, 0, 3, 2, 4))


def make_in_maps(x, cross, Wq, Wk, Wv, mask, cfgs):
    f8 = ml_dtypes.float8_e4m3
    bf = ml_dtypes.bfloat16
    x = np.asarray(x, dtype=np.float32)
    cross = np.asarray(cross, dtype=np.float32)
    mask_np = np.asarray(mask)
    wq_h = np.ascontiguousarray(
        (np.asarray(Wq, np.float32) * SQ).T.reshape(KC, P, DA)
        .transpose(1, 0, 2)).astype(f8)
    wk_h = np.ascontiguousarray(
        (np.asarray(Wk, np.float32) * SQ).T.reshape(KC, P, DA)
        .transpose(1, 0, 2)).astype(f8)
    # Wv x64 split into W0 + R
    wvs = np.asarray(Wv, np.float32) * SW
    w0 = wvs.astype(f8)
    r = (wvs - w0.astype(np.float32)).astype(f8)
    wv0_h = np.ascontiguousarray(
        w0.T.reshape(KC, P, KC, P).transpose(1, 2, 0, 3))
    # r chunk-pairs along dm: [dm, dout] -> [p, do, kcp, 2, 128]
    wvr_h = np.ascontiguousarray(
        r.T.reshape(KC // 2, 2, P, KC, P).transpose(2, 3, 0, 1, 4))

    orders, caps = [], []
    for b in range(B):
        m = mask_np[b].astype(bool)
        un = np.flatnonzero(m)
        ma = np.flatnonzero(~m)
        cap = cfgs[2 * b]["nblk"] * P
        orders.append(np.concatenate([un, ma[:cap - len(un)]]))
        caps.append(cap)
    vpool = []
    for b in range(B):
        m = mask_np[b].astype(bool)
        ma = np.flatnonzero(~m)
        for q in ma[caps[b] - int(m.sum()):]:
            vpool.append((b, int(q)))
    assert len(vpool) == sum(c["vcols"] for c in cfgs)

    in_maps, metas = [], []
    vo = 0
    for c, cfg in enumerate(cfgs):
        b = cfg["batch"]
        blocks = cfg["blocks"]
        kmax, nb = cfg["kmax"], cfg["nb"]
        order = orders[b]
        K = kmax * P
        qpos = np.concatenate([order[g * P:(g + 1) * P] for g in blocks])
        vlist = vpool[vo:vo + cfg["vcols"]]
        vo += cfg["vcols"]

        ct_h = _chunk_hl(cross[b][order[:K]].T, 256)
        xq_h = _chunk_hl(x[b][qpos].T, 256)
        xr = x[b][order[:K]]            # [K, D]
        hi, lo = _hl(xr)
        xhl_h = np.ascontiguousarray(
            np.stack([hi, lo], axis=1).reshape(kmax, P, 2, D)
            .transpose(1, 0, 2, 3))

        qmn_h = np.full((P, len(blocks)), -CBIAS, np.float32)
        dm_h = np.zeros((P, len(blocks) * P), np.float32)
        rows = np.arange(P)
        padded_rows = np.zeros((P, len(blocks)), bool)
        for s, g in enumerate(blocks):
            padded = g * P + rows >= nb
            padded_rows[:, s] = padded
            qmn_h[padded, s] = -np.float32(ESCALE * BIG) - np.float32(CBIAS)
            tri = np.where(rows[None, :] <= rows[:, None], 0.0, -BIG)
            pad_row = np.where(rows[None, :] == rows[:, None], BIG, -BIG)
            dm_h[:, s * P:(s + 1) * P] = np.where(padded[:, None],
                                                  pad_row, tri)
        im = {"ct": ct_h, "xq": xq_h, "wq": wq_h, "wk": wk_h,
              "wv0": wv0_h, "wvr": wvr_h, "xhl": xhl_h, "qmn": qmn_h,
              "dmask": dm_h.astype(bf)}
        if cfg["vcols"]:
            xv_rows = np.stack([x[bb, qq] for bb, qq in vlist])  # [v, D]
            hi, lo = _hl(xv_rows.T)      # [D, v]
            im["xv"] = np.ascontiguousarray(
                np.stack([hi, lo], axis=1).reshape(KC, P, 2, len(vlist))
                .transpose(1, 0, 2, 3))
        in_maps.append(im)
        metas.append({"batch": b, "qpos": qpos, "vlist": vlist,
                      "padded": padded_rows})
    return in_maps, metas


def kernel(x, cross, Wq, Wk, Wv, mask):
    from concourse import bass_utils

    mask_np = np.asarray(mask)
    cfgs, progs = _get_programs(mask_np)
    in_maps, metas = make_in_maps(x, cross, Wq, Wk, Wv, mask, cfgs)

    groups: dict = {}
    for c, cfg in enumerate(cfgs):
        sig = (cfg["kmax"], cfg["blocks"], cfg["vcols"])
        groups.setdefault(sig, []).append(c)

    results = {}
    for sig, cores in groups.items():
        res = bass_utils.run_bass_kernel_spmd(
            progs[sig], [in_maps[c] for c in cores], core_ids=cores)
        for i, c in enumerate(cores):
            results[c] = res.results[i]

    f8 = ml_dtypes.float8_e4m3
    bf = ml_dtypes.bfloat16
    out = np.empty((B, S, D), np.float32)
    for c, meta in enumerate(metas):
        r = results[c]
        o = r["outT"].astype(np.float32).T  # [Z, 1024]
        qa = len(meta["qpos"])
        den = r["den"].astype(np.float32)   # [P, nslots]
        # padded-attn rows: the numerator used fp8(bf16(e')); divide by the
        # identically-rounded denominator so e'/e' cancels exactly
        pad = meta["padded"]
        den = np.where(pad, den.astype(bf).astype(f8).astype(np.float32),
                       den)
        denf = den.T.reshape(-1)            # [QA] slot-major
        out[meta["batch"], meta["qpos"]] = o[:qa] / np.float32(SW) \
            / denf[:, None]
        for i, (bb, qq) in enumerate(meta["vlist"]):
            out[bb, qq] = o[qa + i] / np.float32(SW)
    return out
